# revision 1
# baseline (speedup 1.0000x reference)
"""GCN encoder (2-layer graph conv) on 8 Trainium2 NeuronCores.

Sharding: nodes (rows of x / output) by contiguous 6272-row blocks across the
8 cores; edges partitioned by destination row; 256x256 weights replicated;
per-layer fp16 AllGather of the dense support into lo/hi tables (31/18
dest-tile split keeps table row ids int16); per-128-dest-tile segment-sum as
scaled-one-hot selector matmuls over dma_gathered source rows.

Same sharding as v1 (nodes by dest row, 6272/core; weights replicated;
per-layer fp16 AllGather of the dense support into lo/hi tables; per-128-dest-
tile segment-sum as selector matmuls over dma_gathered source rows).

Perf structure (driven by traces; GpSimd SWDGE descriptor generation for
dma_gather is the critical engine at ~2.9-3.1us per 1024-index op):
  - gathers stream across tile boundaries into two SBUF ring buffers (lo/hi
    source groups), so every dma_gather op is a full 1024 indices instead of
    per-(tile,group) remainders: ~194 ops/layer vs ~280.
  - per-(tile,group) source dedup on the host: repeated sources collapse into
    one gather slot whose selector column carries every edge's (dest,val).
  - gidx is shared by both layers (same graph) and lives in SBUF once.
  - layer-2 dense matmuls interleave into the layer-1 agg loop per tile;
    AG2_lo fires after tile 24, AG2_hi after tile 48, so layer-2 gathers
    start the moment layer-1's finish.
  - AG1_lo fires after the lo-half dense tiles, not after the full loop.
"""

import os
import sys

if "/opt/trn_rl_repo" not in sys.path:
    sys.path.insert(0, "/opt/trn_rl_repo")

import numpy as np

import concourse.mybir as mybir
import concourse.tile as tile
from concourse import bacc, bass_utils
from concourse.bass import ts
from concourse.library_config import mlp

N = 50000
D = 256
NC = 8
P = 128
T = 49
SHARD = T * P  # 6272
NPAD = NC * SHARD  # 50176
LO_T = 31
LO_R = LO_T * P  # 3968
HI_T = T - LO_T  # 18
HI_R = HI_T * P  # 2304
LO_ROWS = NC * LO_R  # 31744 (int16-safe, < 32768)
HI_ROWS = NC * HI_R  # 18432

RING_LO = 64  # ring slots (chunks), lo source group
RING_HI = 32  # ring slots, hi source group
OPC = 8       # chunks per dma_gather op (1024 idxs)

F16 = mybir.dt.float16
F32 = mybir.dt.float32
I16 = mybir.dt.int16

_cache: dict = {}
last_results = None


def _build(c0t: tuple, c1t: tuple):
    key = (c0t, c1t)
    if key in _cache:
        return _cache[key]

    ct = [a + b for a, b in zip(c0t, c1t)]
    off = np.concatenate(([0], np.cumsum(ct))).astype(int)  # sel chunk offsets
    lo_off = np.concatenate(([0], np.cumsum(c0t))).astype(int)
    hi_off = np.concatenate(([0], np.cumsum(c1t))).astype(int)
    TOTC = int(off[-1])
    TOT_LO = int(lo_off[-1])
    TOT_HI = int(hi_off[-1])

    nc = bacc.Bacc(
        "TRN2",
        target_bir_lowering=False,
        debug=False,
        num_devices=NC,
        num_swdge_queues=4,
    )

    xT_d = nc.dram_tensor("xT", [2, P, SHARD], F16, kind="ExternalInput")
    w1_d = nc.dram_tensor("W1h", [2, P, D], F16, kind="ExternalInput")
    w2_d = nc.dram_tensor("W2h", [2, P, D], F16, kind="ExternalInput")
    b1_d = nc.dram_tensor("b1c", [P, 2], F32, kind="ExternalInput")
    b2_d = nc.dram_tensor("b2b", [P, D], F32, kind="ExternalInput")
    # gidx: lo stream then hi stream, shared by both layers
    gidx_d = nc.dram_tensor("gidx", [P, (TOT_LO + TOT_HI) * 8], I16, kind="ExternalInput")
    sel_d = nc.dram_tensor("sel", [P, TOTC * P], F16, kind="ExternalInput")
    out_d = nc.dram_tensor("out", [SHARD, D], F32, kind="ExternalOutput")

    nc.gpsimd.load_library(mlp)

    rg = [list(range(NC))]

    with tile.TileContext(nc) as tc:
        with (
            tc.tile_pool(name="const", bufs=1) as const,
            tc.tile_pool(name="ring", bufs=1) as ring,
            tc.tile_pool(name="spool", bufs=3) as spool,
            tc.tile_pool(name="dense", bufs=3) as dense,
            tc.tile_pool(name="psD", bufs=3, space="PSUM") as psD,
            tc.tile_pool(name="psA", bufs=2, space="PSUM") as psA,
            tc.tile_pool(name="dram", bufs=1, space="DRAM") as dram,
        ):
            cc1_lo = dram.tile([LO_R, D], F16)
            cc1_hi = dram.tile([HI_R, D], F16)
            t1_lo = dram.tile([LO_ROWS, D], F16, addr_space="Shared")
            t1_hi = dram.tile([HI_ROWS, D], F16, addr_space="Shared")
            cc2_lo = dram.tile([LO_R, D], F16)
            cc2_hi = dram.tile([HI_R, D], F16)
            t2_lo = dram.tile([LO_ROWS, D], F16, addr_space="Shared")
            t2_hi = dram.tile([HI_ROWS, D], F16, addr_space="Shared")

            gidx = const.tile([P, (TOT_LO + TOT_HI) * 8], I16)
            nc.sync.dma_start(gidx[:], gidx_d[:])
            b1 = const.tile([P, 2], F32)
            nc.sync.dma_start(b1[:], b1_d[:])
            b2 = const.tile([P, D], F32)
            nc.sync.dma_start(b2[:], b2_d[:])
            w1 = const.tile([P, 2 * D], F16)
            w2 = const.tile([P, 2 * D], F16)
            xsb = const.tile([P, 2 * SHARD], F16, tag="xht", name="xsb")
            hT = const.tile([P, 2 * SHARD], F16, tag="xht", name="hT")
            for h in range(2):
                nc.sync.dma_start(w1[:, h * D : (h + 1) * D], w1_d[h])
                nc.sync.dma_start(w2[:, h * D : (h + 1) * D], w2_d[h])
                nc.sync.dma_start(xsb[:, h * SHARD : (h + 1) * SHARD], xT_d[h])

            glo = ring.tile([P, RING_LO, D], F16, name="glo")
            ghi = ring.tile([P, RING_HI, D], F16, name="ghi")

            def dense_tile(src_sb, w_sb, cc_lo, cc_hi, t):
                ps = psD.tile([P, D], F32, tag="psD", name="ps_dense")
                for h in range(2):
                    nc.tensor.matmul(
                        ps,
                        lhsT=src_sb[:, h * SHARD + t * P : h * SHARD + (t + 1) * P],
                        rhs=w_sb[:, h * D : (h + 1) * D],
                        start=(h == 0),
                        stop=(h == 1),
                    )
                st = dense.tile([P, D], F16, tag="stage", name="stage")
                nc.scalar.copy(st[:], ps[:])
                if t < LO_T:
                    nc.sync.dma_start(cc_lo[ts(t, P), :], st[:])
                else:
                    nc.sync.dma_start(cc_hi[ts(t - LO_T, P), :], st[:])

            def ag(cc, t_out):
                nc.gpsimd.collective_compute(
                    "AllGather",
                    mybir.AluOpType.bypass,
                    replica_groups=rg,
                    ins=[cc.opt()],
                    outs=[t_out.opt()],
                )

            qctr = [0]

            def make_streams(t_lo, t_hi, base8):
                """Per-layer gather emission state. base8: gidx column base."""
                state = {
                    0: {"next": 0, "tot": TOT_LO, "table": t_lo, "ring": glo,
                        "base": base8, "rsz": RING_LO},
                    1: {"next": 0, "tot": TOT_HI, "table": t_hi, "ring": ghi,
                        "base": base8 + TOT_LO * 8, "rsz": RING_HI},
                }

                def ensure(g, upto):
                    s = state[g]
                    while s["next"] < min(upto, s["tot"]):
                        a = s["next"]
                        n = min(OPC, s["tot"] - a)
                        nc.gpsimd.dma_gather(
                            s["ring"][:, a % s["rsz"] : a % s["rsz"] + n, :],
                            s["table"][:],
                            gidx[:, s["base"] + a * 8 : s["base"] + (a + n) * 8],
                            num_idxs=n * P,
                            num_idxs_reg=n * P,
                            elem_size=D,
                            queue_num=qctr[0] % 4,
                        )
                        qctr[0] += 1
                        s["next"] = a + n
                return ensure

            PREF = 1  # prefetch horizon in tiles (RING must cover PREF+1 tiles)

            def sel_tile(t):
                c = ct[t]
                s = spool.tile([P, (max(ct)) * P], F16, tag="sel", name="sel")
                nc.sync.dma_start(
                    s[:, : c * P],
                    sel_d[:, int(off[t]) * P : (int(off[t]) + c) * P],
                )
                return s

            # ---------- layer 1 (+ interleaved layer-2 dense) ----------
            for t in range(LO_T):
                dense_tile(xsb, w1, cc1_lo, cc1_hi, t)
            ag(cc1_lo, t1_lo)
            for t in range(LO_T, T):
                dense_tile(xsb, w1, cc1_lo, cc1_hi, t)
            ag(cc1_hi, t1_hi)

            ensure1 = make_streams(t1_lo, t1_hi, 0)
            for t in range(T):
                tp = min(t + PREF, T - 1)
                ensure1(0, int(lo_off[tp + 1]))
                ensure1(1, int(hi_off[tp + 1]))
                s = sel_tile(t)
                pss = [
                    psA.tile([P, P], F32, tag=f"agg{h}", name=f"agg{h}")
                    for h in range(2)
                ]
                c0, c1, c = c0t[t], c1t[t], ct[t]
                for k in range(c):
                    if k < c0:
                        rng_t, sl = glo, (int(lo_off[t]) + k) % RING_LO
                    else:
                        rng_t, sl = ghi, (int(hi_off[t]) + k - c0) % RING_HI
                    for h in range(2):
                        nc.tensor.matmul(
                            pss[h],
                            lhsT=rng_t[:, sl, h * P : (h + 1) * P],
                            rhs=s[:, k * P : (k + 1) * P],
                            start=(k == 0),
                            stop=(k == c - 1),
                        )
                for h in range(2):
                    nc.scalar.activation(
                        hT[:, h * SHARD + t * P : h * SHARD + (t + 1) * P],
                        pss[h][:],
                        mybir.ActivationFunctionType.Relu,
                        bias=b1[:, h : h + 1],
                    )
                dense_tile(hT, w2, cc2_lo, cc2_hi, t)
                if t == LO_T - 1:
                    ag(cc2_lo, t2_lo)
            ag(cc2_hi, t2_hi)

            # ---------- layer 2 ----------
            ensure2 = make_streams(t2_lo, t2_hi, 0)
            for t in range(T):
                tp = min(t + PREF, T - 1)
                ensure2(0, int(lo_off[tp + 1]))
                ensure2(1, int(hi_off[tp + 1]))
                s = sel_tile(t)
                ps = psD.tile([P, D], F32, tag="psD", name="agg2")
                c0, c1, c = c0t[t], c1t[t], ct[t]
                for k in range(c):
                    if k < c0:
                        rng_t, sl = glo, (int(lo_off[t]) + k) % RING_LO
                    else:
                        rng_t, sl = ghi, (int(hi_off[t]) + k - c0) % RING_HI
                    nc.tensor.matmul(
                        ps,
                        lhsT=s[:, k * P : (k + 1) * P],
                        rhs=rng_t[:, sl, :],
                        start=(k == 0),
                        stop=(k == c - 1),
                    )
                nc.vector.tensor_tensor(ps[:], ps[:], b2[:], mybir.AluOpType.add)
                ot = dense.tile([P, D], F32, tag="ot", name="ot")
                nc.scalar.activation(ot[:], ps[:], mybir.ActivationFunctionType.Relu)
                nc.sync.dma_start(out_d[ts(t, P), :], ot[:])

    nc.compile()
    _cache[key] = nc
    return nc


def _wrap_idx16(flat: np.ndarray) -> np.ndarray:
    """[L] int -> [128, L/16] int16 SBUF wrap: sb[p, s] = flat[s*16 + p%16]."""
    L = flat.shape[0]
    base = flat.reshape(L // 16, 16).T.astype(np.int16)
    return np.tile(base, (8, 1))


def _preprocess(adj_rows, adj_cols, adj_vals):
    r = np.asarray(adj_rows).astype(np.int64)
    c = np.asarray(adj_cols).astype(np.int64)
    v = np.asarray(adj_vals).astype(np.float32)

    core = r // SHARD
    tile_id = (r % SHARD) // P
    dest_local = r % P
    s_core = c // SHARD
    s_loc = c % SHARD
    grp = (s_loc >= LO_R).astype(np.int64)
    idx_local = np.where(grp == 0, s_core * LO_R + s_loc, s_core * HI_R + s_loc - LO_R)

    # dedup repeated sources within each (core, tile, group): one gather slot
    # per unique source; its selector column carries every edge's (dest, val).
    key = ((core * T + tile_id) * 2 + grp) * 60000 + idx_local
    order = np.lexsort((key,))
    key_s = key[order]
    tg_s = key_s // 60000  # (core,tile,grp) id, sorted
    new_slot = np.ones(order.shape[0], np.int64)
    new_slot[1:] = (key_s[1:] != key_s[:-1]).astype(np.int64)
    first_of_tg = np.ones(order.shape[0], bool)
    first_of_tg[1:] = tg_s[1:] != tg_s[:-1]
    new_slot[first_of_tg] = 1
    # slot index within (core,tile,grp) = cumsum of new_slot within group - 1
    cs = np.cumsum(new_slot)
    tg_start_cs = np.zeros(order.shape[0], np.int64)
    start_vals = cs[first_of_tg] - new_slot[first_of_tg]
    tg_ids = np.cumsum(first_of_tg) - 1
    tg_start_cs = start_vals[tg_ids]
    slot = cs - tg_start_cs - 1  # 0-based slot within (core,tile,grp)

    nslots = np.zeros((NC, T, 2), np.int64)
    last_of_tg = np.ones(order.shape[0], bool)
    last_of_tg[:-1] = tg_s[:-1] != tg_s[1:]
    tg_last = tg_s[last_of_tg]
    nslots[(tg_last // 2) // T % NC, (tg_last // 2) % T, tg_last % 2] = slot[last_of_tg] + 1

    c0t = tuple(max(1, -(-int(nslots[:, t, 0].max()) // P)) for t in range(T))
    c1t = tuple(max(1, -(-int(nslots[:, t, 1].max()) // P)) for t in range(T))
    ct = [a + b for a, b in zip(c0t, c1t)]
    off = np.concatenate(([0], np.cumsum(ct))).astype(int)
    lo_off = np.concatenate(([0], np.cumsum(c0t))).astype(int)
    hi_off = np.concatenate(([0], np.cumsum(c1t))).astype(int)
    TOTC = int(off[-1])
    TOT_LO = int(lo_off[-1])
    TOT_HI = int(hi_off[-1])

    core_s = core[order]
    tile_s = tile_id[order]
    grp_s = grp[order]
    dest_s = dest_local[order]
    v_s = v[order]
    idx_s = idx_local[order]

    # global sel chunk id and in-chunk lane for each edge (via its slot)
    chunk_in_tg = slot // P
    lane = slot % P
    gchunk = off[tile_s] + np.where(grp_s == 1, np.asarray(c0t)[tile_s], 0) + chunk_in_tg

    sel = np.zeros((NC, P, TOTC * P), np.float16)
    np.add.at(sel, (core_s, lane, gchunk * P + dest_s), v_s.astype(np.float16))

    # gather index streams: lo chunks of all tiles, then hi chunks
    idx_pad = np.zeros((NC, TOT_LO + TOT_HI, P), np.int16)
    stream_chunk = np.where(
        grp_s == 0,
        lo_off[tile_s] + chunk_in_tg,
        TOT_LO + hi_off[tile_s] + chunk_in_tg,
    )
    idx_pad[core_s, stream_chunk, lane] = idx_s.astype(np.int16)
    gidx = np.zeros((NC, P, (TOT_LO + TOT_HI) * 8), np.int16)
    for cr in range(NC):
        flat = idx_pad[cr].reshape(-1)
        gidx[cr] = _wrap_idx16(flat)

    return c0t, c1t, gidx, sel


def kernel(
    x, adj_rows, adj_cols, adj_vals, pad_n, pos_idx, W1, b1, W2, b2
) -> np.ndarray:
    x = np.asarray(x, np.float32)
    W1 = np.asarray(W1, np.float32)
    b1 = np.asarray(b1, np.float32)
    W2 = np.asarray(W2, np.float32)
    b2 = np.asarray(b2, np.float32)
    pos_idx = np.asarray(pos_idx).astype(np.int64)
    pad_n_i = int(pad_n)
    assert x.shape == (N, D)

    c0t, c1t, gidx, sel = _preprocess(adj_rows, adj_cols, adj_vals)
    nc = _build(c0t, c1t)

    xpad = np.zeros((NPAD, D), np.float32)
    xpad[:N] = x
    w1h = W1.astype(np.float16).reshape(2, P, D)
    w2h = W2.astype(np.float16).reshape(2, P, D)
    b1c = np.ascontiguousarray(b1.reshape(2, P).T.astype(np.float32))
    b2b = np.ascontiguousarray(np.broadcast_to(b2, (P, D)).astype(np.float32))

    in_maps = []
    for cr in range(NC):
        xT = np.ascontiguousarray(
            xpad[cr * SHARD : (cr + 1) * SHARD].T.astype(np.float16).reshape(2, P, SHARD)
        )
        in_maps.append(
            {
                "xT": xT,
                "W1h": w1h,
                "W2h": w2h,
                "b1c": b1c,
                "b2b": b2b,
                "gidx": np.ascontiguousarray(gidx[cr]),
                "sel": np.ascontiguousarray(sel[cr]),
            }
        )

    trace = bool(int(os.environ.get("KERNEL_TRACE", "0")))
    res = None
    for attempt in range(3):
        try:
            res = bass_utils.run_bass_kernel_spmd(
                nc, in_maps, core_ids=list(range(NC)), trace=trace
            )
            break
        except Exception:
            if attempt == 2:
                raise
            import time as _time

            _time.sleep(10.0)
    global last_results
    last_results = res

    h2 = np.concatenate([res.results[cr]["out"] for cr in range(NC)], axis=0)[:N]
    out = np.zeros((pad_n_i, D), np.float32)
    out[pos_idx] = h2
    return out



# revision 12
# speedup vs baseline: 1.0060x; 1.0060x over previous
"""GCN encoder (2-layer graph conv) on 8 Trainium2 NeuronCores.

Sharding: nodes (rows of x / output) by contiguous 6272-row blocks across the
8 cores; edges partitioned by destination row; 256x256 weights replicated;
per-layer fp16 AllGather of the dense support into G=3 source-group tables
(31/9/9 dest-tile split keeps table row ids int16 AND lets the last AllGather
piece be small so the layer-2 transition bubble shrinks); per-128-dest-tile
segment-sum as scaled-one-hot selector matmuls over dma_gathered source rows.

v2 changes (driven by the v1 trace: DMA queues ~78% busy were the limiter,
sel loads alone were ~106MB = ~20% of DMA traffic; GpSimd "busy" was mostly
SWDGE FIFO backpressure, not descriptor-gen compute):
  - selector chunks are generated ON DEVICE by the (idle) Vector engine:
    one tensor_scalar(is_equal, mult) per chunk over a constant iota row,
    with per-partition (dest_id, val) scalar columns uploaded once per graph
    (2 x [128, TOTC] fp16 ~ 0.9MB instead of 106MB of dense selectors).
    This requires one edge per selector lane, so host-side source dedup is
    dropped (+~4% gather indices).
  - G=3 source groups: AllGather pieces fire as their dense/agg tiles
    complete; the last piece is 9 tiles (4.7MB) instead of 18 (9.4MB).
  - initial ring prefetch right after each AllGather piece is issued, so
    gathers start the moment their table lands.
"""

import os
import sys

if "/opt/trn_rl_repo" not in sys.path:
    sys.path.insert(0, "/opt/trn_rl_repo")

import numpy as np

import concourse.mybir as mybir
import concourse.tile as tile
from concourse import bacc, bass_utils
from concourse.bass import ts
from concourse.library_config import mlp

N = 50000
D = 256
NC = 8
P = 128
T = 49
SHARD = T * P  # 6272
NPAD = NC * SHARD  # 50176

G = 3
GTILES = (31, 9, 9)             # dest tiles per source group
GT = (0, 31, 40, 49)            # tile boundaries
GB = (0, 31 * P, 40 * P, 49 * P)  # per-core row boundaries (3968, 5120, 6272)
GR = (31 * P, 9 * P, 9 * P)     # rows/core per group
TROWS = tuple(NC * r for r in GR)  # table rows: 31744, 9216, 9216 (< 32768)

RINGS = (64, 24, 24)  # ring slots (chunks) per source group; must exceed the
                      # in-flight window (2 tiles of chunks + OPC overshoot)
OPC = 8               # chunks per dma_gather op (1024 idxs)

F16 = mybir.dt.float16
F32 = mybir.dt.float32
I16 = mybir.dt.int16

# gather-table dtype: F16 (safe) or mybir.dt.float8e4 (halves gather/AG
# bytes; raises rel err ~50x, still under the 2e-2 gate -- flip after A/B).
TAB = F16
TAB_BYTES = 2 if TAB == F16 else 1

_cache: dict = {}
last_results = None


def _build(cgt: tuple):
    """cgt: T-tuple of G-tuples of per-(tile, group) chunk counts."""
    key = cgt
    if key in _cache:
        return _cache[key]

    ct = [sum(c) for c in cgt]
    off = np.concatenate(([0], np.cumsum(ct))).astype(int)  # sel chunk offsets
    goff = []  # per-group cumulative chunk stream offsets over tiles
    for g in range(G):
        goff.append(
            np.concatenate(([0], np.cumsum([cgt[t][g] for t in range(T)]))).astype(int)
        )
    TOTC = int(off[-1])
    TOTG = [int(goff[g][-1]) for g in range(G)]
    TOT = sum(TOTG)
    GBASE = [8 * sum(TOTG[:g]) for g in range(G)]  # gidx column bases
    MAXCT = max(ct)

    nc = bacc.Bacc(
        "TRN2",
        target_bir_lowering=False,
        debug=False,
        num_devices=NC,
        num_swdge_queues=4,
    )

    xT_d = nc.dram_tensor("xT", [2, P, SHARD], F16, kind="ExternalInput")
    w1_d = nc.dram_tensor("W1h", [2, P, D], F16, kind="ExternalInput")
    w2_d = nc.dram_tensor("W2h", [2, P, D], F16, kind="ExternalInput")
    b1_d = nc.dram_tensor("b1c", [P, 2], F32, kind="ExternalInput")
    b2_d = nc.dram_tensor("b2b", [P, D], F32, kind="ExternalInput")
    gidx_d = nc.dram_tensor("gidx", [P, TOT * 8], I16, kind="ExternalInput")
    dst_d = nc.dram_tensor("dst", [P, TOTC], F16, kind="ExternalInput")
    val_d = nc.dram_tensor("val", [P, TOTC], F16, kind="ExternalInput")
    out_d = nc.dram_tensor("out", [SHARD, D], F32, kind="ExternalOutput")

    nc.gpsimd.load_library(mlp)

    rg = [list(range(NC))]

    with tile.TileContext(nc) as tc:
        with (
            tc.tile_pool(name="const", bufs=1) as const,
            tc.tile_pool(name="ring", bufs=1) as ring,
            tc.tile_pool(name="spool", bufs=3) as spool,
            tc.tile_pool(name="dense", bufs=3) as dense,
            tc.tile_pool(name="psD", bufs=3, space="PSUM") as psD,
            tc.tile_pool(name="psA", bufs=2, space="PSUM") as psA,
            tc.tile_pool(name="dram", bufs=1, space="DRAM") as dram,
        ):
            cc1 = [dram.tile([GR[g], D], TAB, name=f"cc1_{g}") for g in range(G)]
            t1 = [
                dram.tile([TROWS[g], D], TAB, addr_space="Shared", name=f"t1_{g}")
                for g in range(G)
            ]
            cc2 = [dram.tile([GR[g], D], TAB, name=f"cc2_{g}") for g in range(G)]
            t2 = [
                dram.tile([TROWS[g], D], TAB, addr_space="Shared", name=f"t2_{g}")
                for g in range(G)
            ]

            gidx = const.tile([P, TOT * 8], I16)
            nc.sync.dma_start(gidx[:], gidx_d[:])
            dst = const.tile([P, TOTC], F16)
            nc.sync.dma_start(dst[:], dst_d[:])
            val = const.tile([P, TOTC], F16)
            nc.sync.dma_start(val[:], val_d[:])
            b1 = const.tile([P, 2], F32)
            nc.sync.dma_start(b1[:], b1_d[:])
            b2 = const.tile([P, D], F32)
            nc.sync.dma_start(b2[:], b2_d[:])
            w1 = const.tile([P, 2 * D], F16)
            w2 = const.tile([P, 2 * D], F16)
            xsb = const.tile([P, 2 * SHARD], F16, tag="xht", name="xsb")
            hT = const.tile([P, 2 * SHARD], F16, tag="xht", name="hT")
            for h in range(2):
                nc.sync.dma_start(w1[:, h * D : (h + 1) * D], w1_d[h])
                nc.sync.dma_start(w2[:, h * D : (h + 1) * D], w2_d[h])
                nc.sync.dma_start(xsb[:, h * SHARD : (h + 1) * SHARD], xT_d[h])

            iot = const.tile([P, P], F16, name="iota")
            nc.gpsimd.iota(
                iot[:],
                [[1, P]],
                channel_multiplier=0,
                allow_small_or_imprecise_dtypes=True,
            )

            rings = [
                ring.tile([P, RINGS[g], D], TAB, name=f"ring{g}") for g in range(G)
            ]

            # per-tile matmul consumption map: k -> (group, chunk-in-stream)
            kmap = []
            for t in range(T):
                m = []
                for g in range(G):
                    for j in range(cgt[t][g]):
                        m.append((g, int(goff[g][t]) + j))
                kmap.append(m)

            def dense_tile(src_sb, w_sb, cc, t):
                ps = psD.tile([P, D], F32, tag="psD", name="ps_dense")
                for h in range(2):
                    nc.tensor.matmul(
                        ps,
                        lhsT=src_sb[:, h * SHARD + t * P : h * SHARD + (t + 1) * P],
                        rhs=w_sb[:, h * D : (h + 1) * D],
                        start=(h == 0),
                        stop=(h == 1),
                    )
                st = dense.tile([P, D], TAB, tag="stage", name="stage")
                nc.scalar.copy(st[:], ps[:])
                g = next(gg for gg in range(G) if GT[gg] <= t < GT[gg + 1])
                nc.sync.dma_start(cc[g][ts(t - GT[g], P), :], st[:])

            def ag(cc_g, t_out):
                nc.gpsimd.collective_compute(
                    "AllGather",
                    mybir.AluOpType.bypass,
                    replica_groups=rg,
                    ins=[cc_g.opt()],
                    outs=[t_out.opt()],
                )

            qctr = [0]

            def make_streams(tabs):
                state = [
                    {"next": 0, "tot": TOTG[g], "table": tabs[g], "ring": rings[g],
                     "base": GBASE[g], "rsz": RINGS[g]}
                    for g in range(G)
                ]

                def ensure(g, upto):
                    s = state[g]
                    while s["next"] < min(upto, s["tot"]):
                        a = s["next"]
                        n = min(OPC, s["tot"] - a)
                        nc.gpsimd.dma_gather(
                            s["ring"][:, a % s["rsz"] : a % s["rsz"] + n, :],
                            s["table"][:],
                            gidx[:, s["base"] + a * 8 : s["base"] + (a + n) * 8],
                            num_idxs=n * P,
                            num_idxs_reg=n * P,
                            elem_size=D,
                            queue_num=qctr[0] % 4,
                        )
                        qctr[0] += 1
                        s["next"] = a + n
                return ensure

            PREF = 1  # prefetch horizon in tiles

            def sel_tile(t):
                c = ct[t]
                o = int(off[t])
                s = spool.tile([P, MAXCT, P], F16, tag="sel", name="sel")
                nc.vector.tensor_tensor(
                    s[:, :c, :],
                    iot[:, None, :].broadcast_to([P, c, P]),
                    dst[:, o : o + c, None].broadcast_to([P, c, P]),
                    mybir.AluOpType.is_equal,
                )
                nc.vector.tensor_tensor(
                    s[:, :c, :],
                    s[:, :c, :],
                    val[:, o : o + c, None].broadcast_to([P, c, P]),
                    mybir.AluOpType.mult,
                )
                return s

            # ---------- layer 1 dense (+ pipelined AllGather pieces) -------
            ensure1 = make_streams(t1)
            for t in range(T):
                dense_tile(xsb, w1, cc1, t)
                for g in range(G):
                    if t == GT[g + 1] - 1:
                        ag(cc1[g], t1[g])
                        ensure1(g, RINGS[g])  # prefetch as soon as table lands

            # ---------- layer 1 agg (+ interleaved layer-2 dense) ----------
            for t in range(T):
                tp = min(t + PREF, T - 1)
                for g in range(G):
                    ensure1(g, int(goff[g][tp + 1]))
                s = sel_tile(t)
                pss = [
                    psA.tile([P, P], F32, tag=f"agg{h}", name=f"agg{h}")
                    for h in range(2)
                ]
                c = ct[t]
                for k, (g, j) in enumerate(kmap[t]):
                    sl = j % RINGS[g]
                    for h in range(2):
                        nc.tensor.matmul(
                            pss[h],
                            lhsT=rings[g][:, sl, h * P : (h + 1) * P],
                            rhs=s[:, k, :],
                            start=(k == 0),
                            stop=(k == c - 1),
                        )
                for h in range(2):
                    nc.scalar.activation(
                        hT[:, h * SHARD + t * P : h * SHARD + (t + 1) * P],
                        pss[h][:],
                        mybir.ActivationFunctionType.Relu,
                        bias=b1[:, h : h + 1],
                    )
                dense_tile(hT, w2, cc2, t)
                for g in range(G):
                    if t == GT[g + 1] - 1:
                        ag(cc2[g], t2[g])

            # ---------- layer 2 ----------
            ensure2 = make_streams(t2)
            for t in range(T):
                tp = min(t + PREF, T - 1)
                for g in range(G):
                    ensure2(g, int(goff[g][tp + 1]))
                s = sel_tile(t)
                ps = psD.tile([P, D], F32, tag="psD", name="agg2")
                c = ct[t]
                for k, (g, j) in enumerate(kmap[t]):
                    sl = j % RINGS[g]
                    nc.tensor.matmul(
                        ps,
                        lhsT=s[:, k, :],
                        rhs=rings[g][:, sl, :],
                        start=(k == 0),
                        stop=(k == c - 1),
                    )
                nc.vector.tensor_tensor(ps[:], ps[:], b2[:], mybir.AluOpType.add)
                ot = dense.tile([P, D], F32, tag="ot", name="ot")
                nc.scalar.activation(ot[:], ps[:], mybir.ActivationFunctionType.Relu)
                nc.sync.dma_start(out_d[ts(t, P), :], ot[:])

    nc.compile()
    _cache[key] = nc
    return nc


def _wrap_idx16(flat: np.ndarray) -> np.ndarray:
    """[L] int -> [128, L/16] int16 SBUF wrap: sb[p, s] = flat[s*16 + p%16]."""
    L = flat.shape[0]
    base = flat.reshape(L // 16, 16).T.astype(np.int16)
    return np.tile(base, (8, 1))


def _preprocess(adj_rows, adj_cols, adj_vals):
    r = np.asarray(adj_rows).astype(np.int64)
    c = np.asarray(adj_cols).astype(np.int64)
    v = np.asarray(adj_vals).astype(np.float32)

    core = r // SHARD
    tile_id = (r % SHARD) // P
    dest_local = r % P
    s_core = c // SHARD
    s_loc = c % SHARD
    grp = (s_loc >= GB[1]).astype(np.int64) + (s_loc >= GB[2]).astype(np.int64)
    idx_local = s_core * np.asarray(GR)[grp] + (s_loc - np.asarray(GB)[grp])

    # slot assignment: rank within each (core, tile, group), edge order
    key = ((core * T + tile_id) * G + grp)
    order = np.argsort(key, kind="stable")
    key_s = key[order]
    first = np.ones(order.shape[0], bool)
    first[1:] = key_s[1:] != key_s[:-1]
    idx = np.arange(order.shape[0])
    start = idx[first]
    gid = np.cumsum(first) - 1
    slot = idx - start[gid]

    cnt = np.zeros(NC * T * G, np.int64)
    np.add.at(cnt, key, 1)
    cnt = cnt.reshape(NC, T, G)
    cgt = tuple(
        tuple(max(1, -(-int(cnt[:, t, g].max()) // P)) for g in range(G))
        for t in range(T)
    )

    ct = [sum(cg) for cg in cgt]
    off = np.concatenate(([0], np.cumsum(ct))).astype(int)
    goff = []
    for g in range(G):
        goff.append(
            np.concatenate(([0], np.cumsum([cgt[t][g] for t in range(T)]))).astype(int)
        )
    TOTC = int(off[-1])
    TOTG = [int(goff[g][-1]) for g in range(G)]
    TOT = sum(TOTG)

    core_s = core[order]
    tile_s = tile_id[order]
    grp_s = grp[order]
    dest_s = dest_local[order]
    v_s = v[order]
    idx_s = idx_local[order]

    chunk_in_tg = slot // P
    lane = slot % P
    # sel column: off[t] + chunks of lower groups + chunk_in_tg
    gprefix = np.zeros((T, G), np.int64)
    for t in range(T):
        acc = 0
        for g in range(G):
            gprefix[t, g] = acc
            acc += cgt[t][g]
    scol = off[tile_s] + gprefix[tile_s, grp_s] + chunk_in_tg

    dst = np.zeros((NC, P, TOTC), np.float16)
    valq = np.zeros((NC, P, TOTC), np.float16)
    dst[core_s, lane, scol] = dest_s.astype(np.float16)
    valq[core_s, lane, scol] = v_s.astype(np.float16)

    # gather index streams, group-concatenated
    stream_base = np.asarray([sum(TOTG[:g]) for g in range(G)])
    stream_chunk = stream_base[grp_s] + goff_lookup(goff, tile_s, grp_s) + chunk_in_tg
    idx_pad = np.zeros((NC, TOT, P), np.int16)
    idx_pad[core_s, stream_chunk, lane] = idx_s.astype(np.int16)
    gidx = np.zeros((NC, P, TOT * 8), np.int16)
    for cr in range(NC):
        gidx[cr] = _wrap_idx16(idx_pad[cr].reshape(-1))

    return cgt, gidx, dst, valq


def goff_lookup(goff, tile_s, grp_s):
    tab = np.stack([goff[g][:T] for g in range(G)], axis=1)  # [T, G]
    return tab[tile_s, grp_s]


def kernel(
    x, adj_rows, adj_cols, adj_vals, pad_n, pos_idx, W1, b1, W2, b2
) -> np.ndarray:
    x = np.asarray(x, np.float32)
    W1 = np.asarray(W1, np.float32)
    b1 = np.asarray(b1, np.float32)
    W2 = np.asarray(W2, np.float32)
    b2 = np.asarray(b2, np.float32)
    pos_idx = np.asarray(pos_idx).astype(np.int64)
    pad_n_i = int(pad_n)
    assert x.shape == (N, D)

    cgt, gidx, dstq, valq = _preprocess(adj_rows, adj_cols, adj_vals)
    nc = _build(cgt)

    xpad = np.zeros((NPAD, D), np.float32)
    xpad[:N] = x
    w1h = W1.astype(np.float16).reshape(2, P, D)
    w2h = W2.astype(np.float16).reshape(2, P, D)
    b1c = np.ascontiguousarray(b1.reshape(2, P).T.astype(np.float32))
    b2b = np.ascontiguousarray(np.broadcast_to(b2, (P, D)).astype(np.float32))

    in_maps = []
    for cr in range(NC):
        xT = np.ascontiguousarray(
            xpad[cr * SHARD : (cr + 1) * SHARD].T.astype(np.float16).reshape(2, P, SHARD)
        )
        in_maps.append(
            {
                "xT": xT,
                "W1h": w1h,
                "W2h": w2h,
                "b1c": b1c,
                "b2b": b2b,
                "gidx": np.ascontiguousarray(gidx[cr]),
                "dst": np.ascontiguousarray(dstq[cr]),
                "val": np.ascontiguousarray(valq[cr]),
            }
        )

    trace = bool(int(os.environ.get("KERNEL_TRACE", "0")))
    res = None
    for attempt in range(3):
        try:
            res = bass_utils.run_bass_kernel_spmd(
                nc, in_maps, core_ids=list(range(NC)), trace=trace
            )
            break
        except Exception:
            if attempt == 2:
                raise
            import time as _time

            _time.sleep(10.0)
    global last_results
    last_results = res

    h2 = np.concatenate([res.results[cr]["out"] for cr in range(NC)], axis=0)[:N]
    out = np.zeros((pad_n_i, D), np.float32)
    out[pos_idx] = h2
    return out


# revision 16
# speedup vs baseline: 1.0133x; 1.0073x over previous
"""GCN encoder (2-layer graph conv) on 8 Trainium2 NeuronCores.

Sharding: nodes (rows of x / output) by contiguous 6272-row blocks across the
8 cores; edges partitioned by destination row; 256x256 weights replicated;
per-layer fp16 AllGather of the dense support into G=3 source-group tables
(31/9/9 dest-tile split keeps table row ids int16 AND lets the last AllGather
piece be small so the layer-2 transition bubble shrinks); per-128-dest-tile
segment-sum as scaled-one-hot selector matmuls over dma_gathered source rows.

v2 changes (driven by the v1 trace: DMA queues ~78% busy were the limiter,
sel loads alone were ~106MB = ~20% of DMA traffic; GpSimd "busy" was mostly
SWDGE FIFO backpressure, not descriptor-gen compute):
  - selector chunks are generated ON DEVICE by the (idle) Vector engine:
    one tensor_scalar(is_equal, mult) per chunk over a constant iota row,
    with per-partition (dest_id, val) scalar columns uploaded once per graph
    (2 x [128, TOTC] fp16 ~ 0.9MB instead of 106MB of dense selectors).
    This requires one edge per selector lane, so host-side source dedup is
    dropped (+~4% gather indices).
  - G=3 source groups: AllGather pieces fire as their dense/agg tiles
    complete; the last piece is 9 tiles (4.7MB) instead of 18 (9.4MB).
  - initial ring prefetch right after each AllGather piece is issued, so
    gathers start the moment their table lands.
"""

import os
import sys

if "/opt/trn_rl_repo" not in sys.path:
    sys.path.insert(0, "/opt/trn_rl_repo")

import numpy as np

import concourse.mybir as mybir
import concourse.tile as tile
from concourse import bacc, bass_utils
from concourse.bass import ts
from concourse.library_config import mlp

N = 50000
D = 256
NC = 8
P = 128
T = 49
SHARD = T * P  # 6272
NPAD = NC * SHARD  # 50176

G = 3
GTILES = (31, 9, 9)             # dest tiles per source group
GT = (0, 31, 40, 49)            # tile boundaries
GB = (0, 31 * P, 40 * P, 49 * P)  # per-core row boundaries (3968, 5120, 6272)
GR = (31 * P, 9 * P, 9 * P)     # rows/core per group
TROWS = tuple(NC * r for r in GR)  # table rows: 31744, 9216, 9216 (< 32768)

RINGS = (96, 40, 40)  # ring slots (chunks) per source group; must comfortably
                      # exceed the in-flight window ((PREF+1) tiles of chunks +
                      # OPC overshoot) or gathers serialize against PE reads
OPC = 8               # chunks per dma_gather op (1024 idxs)

F16 = mybir.dt.float16
F32 = mybir.dt.float32
I16 = mybir.dt.int16

# gather-table dtype: F16 (safe) or mybir.dt.float8e4 (halves gather/AG
# bytes; raises rel err ~50x, still under the 2e-2 gate -- flip after A/B).
TAB = F16
TAB_BYTES = 2 if TAB == F16 else 1

_cache: dict = {}
last_results = None


def _build(cgt: tuple):
    """cgt: T-tuple of G-tuples of per-(tile, group) chunk counts."""
    key = cgt
    if key in _cache:
        return _cache[key]

    ct = [sum(c) for c in cgt]
    off = np.concatenate(([0], np.cumsum(ct))).astype(int)  # sel chunk offsets
    goff = []  # per-group cumulative chunk stream offsets over tiles
    for g in range(G):
        goff.append(
            np.concatenate(([0], np.cumsum([cgt[t][g] for t in range(T)]))).astype(int)
        )
    TOTC = int(off[-1])
    TOTG = [int(goff[g][-1]) for g in range(G)]
    TOT = sum(TOTG)
    GBASE = [8 * sum(TOTG[:g]) for g in range(G)]  # gidx column bases
    MAXCT = max(ct)

    nc = bacc.Bacc(
        "TRN2",
        target_bir_lowering=False,
        debug=False,
        num_devices=NC,
        num_swdge_queues=4,
    )

    xT_d = nc.dram_tensor("xT", [2, P, SHARD], F16, kind="ExternalInput")
    w1_d = nc.dram_tensor("W1h", [2, P, D], F16, kind="ExternalInput")
    w2_d = nc.dram_tensor("W2h", [2, P, D], F16, kind="ExternalInput")
    b1_d = nc.dram_tensor("b1c", [P, 2], F32, kind="ExternalInput")
    b2_d = nc.dram_tensor("b2b", [P, D], F32, kind="ExternalInput")
    gidx_d = nc.dram_tensor("gidx", [P, TOT * 8], I16, kind="ExternalInput")
    dst_d = nc.dram_tensor("dst", [P, TOTC], F16, kind="ExternalInput")
    val_d = nc.dram_tensor("val", [P, TOTC], F16, kind="ExternalInput")
    out_d = nc.dram_tensor("out", [SHARD, D], F32, kind="ExternalOutput")

    nc.gpsimd.load_library(mlp)

    rg = [list(range(NC))]

    with tile.TileContext(nc) as tc:
        with (
            tc.tile_pool(name="const", bufs=1) as const,
            tc.tile_pool(name="ring", bufs=1) as ring,
            tc.tile_pool(name="spool", bufs=3) as spool,
            tc.tile_pool(name="dense", bufs=3) as dense,
            tc.tile_pool(name="psD", bufs=3, space="PSUM") as psD,
            tc.tile_pool(name="psA", bufs=2, space="PSUM") as psA,
            tc.tile_pool(name="dram", bufs=1, space="DRAM") as dram,
        ):
            cc1 = [dram.tile([GR[g], D], TAB, name=f"cc1_{g}") for g in range(G)]
            t1 = [
                dram.tile([TROWS[g], D], TAB, addr_space="Shared", name=f"t1_{g}")
                for g in range(G)
            ]
            cc2 = [dram.tile([GR[g], D], TAB, name=f"cc2_{g}") for g in range(G)]
            t2 = [
                dram.tile([TROWS[g], D], TAB, addr_space="Shared", name=f"t2_{g}")
                for g in range(G)
            ]

            gidx = const.tile([P, TOT * 8], I16)
            nc.sync.dma_start(gidx[:], gidx_d[:])
            dst = const.tile([P, TOTC], F16)
            nc.sync.dma_start(dst[:], dst_d[:])
            val = const.tile([P, TOTC], F16)
            nc.sync.dma_start(val[:], val_d[:])
            b1 = const.tile([P, 2], F32)
            nc.sync.dma_start(b1[:], b1_d[:])
            b2 = const.tile([P, D], F32)
            nc.sync.dma_start(b2[:], b2_d[:])
            w1 = const.tile([P, 2 * D], F16)
            w2 = const.tile([P, 2 * D], F16)
            xsb = const.tile([P, 2 * SHARD], F16, tag="xht", name="xsb")
            hT = const.tile([P, 2 * SHARD], F16, tag="xht", name="hT")
            for h in range(2):
                nc.sync.dma_start(w1[:, h * D : (h + 1) * D], w1_d[h])
                nc.sync.dma_start(w2[:, h * D : (h + 1) * D], w2_d[h])
                nc.sync.dma_start(xsb[:, h * SHARD : (h + 1) * SHARD], xT_d[h])

            iot = const.tile([P, P], F16, name="iota")
            nc.gpsimd.iota(
                iot[:],
                [[1, P]],
                channel_multiplier=0,
                allow_small_or_imprecise_dtypes=True,
            )

            rings = [
                ring.tile([P, RINGS[g], D], TAB, name=f"ring{g}") for g in range(G)
            ]

            # per-tile matmul consumption map: k -> (group, chunk-in-stream)
            kmap = []
            for t in range(T):
                m = []
                for g in range(G):
                    for j in range(cgt[t][g]):
                        m.append((g, int(goff[g][t]) + j))
                kmap.append(m)

            def dense_tile(src_sb, w_sb, cc, t):
                ps = psD.tile([P, D], F32, tag="psD", name="ps_dense")
                for h in range(2):
                    nc.tensor.matmul(
                        ps,
                        lhsT=src_sb[:, h * SHARD + t * P : h * SHARD + (t + 1) * P],
                        rhs=w_sb[:, h * D : (h + 1) * D],
                        start=(h == 0),
                        stop=(h == 1),
                    )
                st = dense.tile([P, D], TAB, tag="stage", name="stage")
                nc.scalar.copy(st[:], ps[:])
                g = next(gg for gg in range(G) if GT[gg] <= t < GT[gg + 1])
                nc.sync.dma_start(cc[g][ts(t - GT[g], P), :], st[:])

            def ag(cc_g, t_out):
                nc.gpsimd.collective_compute(
                    "AllGather",
                    mybir.AluOpType.bypass,
                    replica_groups=rg,
                    ins=[cc_g.opt()],
                    outs=[t_out.opt()],
                )

            qctr = [0]

            def make_streams(tabs):
                state = [
                    {"next": 0, "tot": TOTG[g], "table": tabs[g], "ring": rings[g],
                     "base": GBASE[g], "rsz": RINGS[g]}
                    for g in range(G)
                ]

                def ensure(g, upto):
                    s = state[g]
                    while s["next"] < min(upto, s["tot"]):
                        a = s["next"]
                        n = min(OPC, s["tot"] - a)
                        nc.gpsimd.dma_gather(
                            s["ring"][:, a % s["rsz"] : a % s["rsz"] + n, :],
                            s["table"][:],
                            gidx[:, s["base"] + a * 8 : s["base"] + (a + n) * 8],
                            num_idxs=n * P,
                            num_idxs_reg=n * P,
                            elem_size=D,
                            queue_num=qctr[0] % 4,
                        )
                        qctr[0] += 1
                        s["next"] = a + n
                return ensure

            PREF = 2  # prefetch horizon in tiles

            def sel_tile(t):
                c = ct[t]
                o = int(off[t])
                s = spool.tile([P, MAXCT, P], F16, tag="sel", name="sel")
                nc.vector.tensor_tensor(
                    s[:, :c, :],
                    iot[:, None, :].broadcast_to([P, c, P]),
                    dst[:, o : o + c, None].broadcast_to([P, c, P]),
                    mybir.AluOpType.is_equal,
                )
                nc.vector.tensor_tensor(
                    s[:, :c, :],
                    s[:, :c, :],
                    val[:, o : o + c, None].broadcast_to([P, c, P]),
                    mybir.AluOpType.mult,
                )
                return s

            # ---------- layer 1 dense (+ pipelined AllGather pieces) -------
            ensure1 = make_streams(t1)
            for t in range(T):
                dense_tile(xsb, w1, cc1, t)
                for g in range(G):
                    if t == GT[g + 1] - 1:
                        ag(cc1[g], t1[g])
                        ensure1(g, RINGS[g])  # prefetch as soon as table lands

            # ---------- layer 1 agg (+ interleaved layer-2 dense) ----------
            for t in range(T):
                tp = min(t + PREF, T - 1)
                for g in range(G):
                    ensure1(g, int(goff[g][tp + 1]))
                s = sel_tile(t)
                pss = [
                    psA.tile([P, P], F32, tag=f"agg{h}", name=f"agg{h}")
                    for h in range(2)
                ]
                c = ct[t]
                for k, (g, j) in enumerate(kmap[t]):
                    sl = j % RINGS[g]
                    for h in range(2):
                        nc.tensor.matmul(
                            pss[h],
                            lhsT=rings[g][:, sl, h * P : (h + 1) * P],
                            rhs=s[:, k, :],
                            start=(k == 0),
                            stop=(k == c - 1),
                        )
                for h in range(2):
                    nc.scalar.activation(
                        hT[:, h * SHARD + t * P : h * SHARD + (t + 1) * P],
                        pss[h][:],
                        mybir.ActivationFunctionType.Relu,
                        bias=b1[:, h : h + 1],
                    )
                dense_tile(hT, w2, cc2, t)
                for g in range(G):
                    if t == GT[g + 1] - 1:
                        ag(cc2[g], t2[g])

            # ---------- layer 2 ----------
            ensure2 = make_streams(t2)
            for t in range(T):
                tp = min(t + PREF, T - 1)
                for g in range(G):
                    ensure2(g, int(goff[g][tp + 1]))
                s = sel_tile(t)
                ps = psD.tile([P, D], F32, tag="psD", name="agg2")
                c = ct[t]
                for k, (g, j) in enumerate(kmap[t]):
                    sl = j % RINGS[g]
                    nc.tensor.matmul(
                        ps,
                        lhsT=s[:, k, :],
                        rhs=rings[g][:, sl, :],
                        start=(k == 0),
                        stop=(k == c - 1),
                    )
                nc.vector.tensor_tensor(ps[:], ps[:], b2[:], mybir.AluOpType.add)
                ot = dense.tile([P, D], F32, tag="ot", name="ot")
                nc.scalar.activation(ot[:], ps[:], mybir.ActivationFunctionType.Relu)
                nc.sync.dma_start(out_d[ts(t, P), :], ot[:])

    nc.compile()
    _cache[key] = nc
    return nc


def _wrap_idx16(flat: np.ndarray) -> np.ndarray:
    """[L] int -> [128, L/16] int16 SBUF wrap: sb[p, s] = flat[s*16 + p%16]."""
    L = flat.shape[0]
    base = flat.reshape(L // 16, 16).T.astype(np.int16)
    return np.tile(base, (8, 1))


def _preprocess(adj_rows, adj_cols, adj_vals):
    r = np.asarray(adj_rows).astype(np.int64)
    c = np.asarray(adj_cols).astype(np.int64)
    v = np.asarray(adj_vals).astype(np.float32)

    core = r // SHARD
    tile_id = (r % SHARD) // P
    dest_local = r % P
    s_core = c // SHARD
    s_loc = c % SHARD
    grp = (s_loc >= GB[1]).astype(np.int64) + (s_loc >= GB[2]).astype(np.int64)
    idx_local = s_core * np.asarray(GR)[grp] + (s_loc - np.asarray(GB)[grp])

    # slot assignment: rank within each (core, tile, group), edge order
    key = ((core * T + tile_id) * G + grp)
    order = np.argsort(key, kind="stable")
    key_s = key[order]
    first = np.ones(order.shape[0], bool)
    first[1:] = key_s[1:] != key_s[:-1]
    idx = np.arange(order.shape[0])
    start = idx[first]
    gid = np.cumsum(first) - 1
    slot = idx - start[gid]

    cnt = np.zeros(NC * T * G, np.int64)
    np.add.at(cnt, key, 1)
    cnt = cnt.reshape(NC, T, G)
    cgt = tuple(
        tuple(max(1, -(-int(cnt[:, t, g].max()) // P)) for g in range(G))
        for t in range(T)
    )

    ct = [sum(cg) for cg in cgt]
    off = np.concatenate(([0], np.cumsum(ct))).astype(int)
    goff = []
    for g in range(G):
        goff.append(
            np.concatenate(([0], np.cumsum([cgt[t][g] for t in range(T)]))).astype(int)
        )
    TOTC = int(off[-1])
    TOTG = [int(goff[g][-1]) for g in range(G)]
    TOT = sum(TOTG)

    core_s = core[order]
    tile_s = tile_id[order]
    grp_s = grp[order]
    dest_s = dest_local[order]
    v_s = v[order]
    idx_s = idx_local[order]

    chunk_in_tg = slot // P
    lane = slot % P
    # sel column: off[t] + chunks of lower groups + chunk_in_tg
    gprefix = np.zeros((T, G), np.int64)
    for t in range(T):
        acc = 0
        for g in range(G):
            gprefix[t, g] = acc
            acc += cgt[t][g]
    scol = off[tile_s] + gprefix[tile_s, grp_s] + chunk_in_tg

    dst = np.zeros((NC, P, TOTC), np.float16)
    valq = np.zeros((NC, P, TOTC), np.float16)
    dst[core_s, lane, scol] = dest_s.astype(np.float16)
    valq[core_s, lane, scol] = v_s.astype(np.float16)

    # gather index streams, group-concatenated
    stream_base = np.asarray([sum(TOTG[:g]) for g in range(G)])
    stream_chunk = stream_base[grp_s] + goff_lookup(goff, tile_s, grp_s) + chunk_in_tg
    idx_pad = np.zeros((NC, TOT, P), np.int16)
    idx_pad[core_s, stream_chunk, lane] = idx_s.astype(np.int16)
    gidx = np.zeros((NC, P, TOT * 8), np.int16)
    for cr in range(NC):
        gidx[cr] = _wrap_idx16(idx_pad[cr].reshape(-1))

    return cgt, gidx, dst, valq


def goff_lookup(goff, tile_s, grp_s):
    tab = np.stack([goff[g][:T] for g in range(G)], axis=1)  # [T, G]
    return tab[tile_s, grp_s]


def kernel(
    x, adj_rows, adj_cols, adj_vals, pad_n, pos_idx, W1, b1, W2, b2
) -> np.ndarray:
    x = np.asarray(x, np.float32)
    W1 = np.asarray(W1, np.float32)
    b1 = np.asarray(b1, np.float32)
    W2 = np.asarray(W2, np.float32)
    b2 = np.asarray(b2, np.float32)
    pos_idx = np.asarray(pos_idx).astype(np.int64)
    pad_n_i = int(pad_n)
    assert x.shape == (N, D)

    cgt, gidx, dstq, valq = _preprocess(adj_rows, adj_cols, adj_vals)
    nc = _build(cgt)

    xpad = np.zeros((NPAD, D), np.float32)
    xpad[:N] = x
    w1h = W1.astype(np.float16).reshape(2, P, D)
    w2h = W2.astype(np.float16).reshape(2, P, D)
    b1c = np.ascontiguousarray(b1.reshape(2, P).T.astype(np.float32))
    b2b = np.ascontiguousarray(np.broadcast_to(b2, (P, D)).astype(np.float32))

    in_maps = []
    for cr in range(NC):
        xT = np.ascontiguousarray(
            xpad[cr * SHARD : (cr + 1) * SHARD].T.astype(np.float16).reshape(2, P, SHARD)
        )
        in_maps.append(
            {
                "xT": xT,
                "W1h": w1h,
                "W2h": w2h,
                "b1c": b1c,
                "b2b": b2b,
                "gidx": np.ascontiguousarray(gidx[cr]),
                "dst": np.ascontiguousarray(dstq[cr]),
                "val": np.ascontiguousarray(valq[cr]),
            }
        )

    trace = bool(int(os.environ.get("KERNEL_TRACE", "0")))
    res = None
    for attempt in range(3):
        try:
            res = bass_utils.run_bass_kernel_spmd(
                nc, in_maps, core_ids=list(range(NC)), trace=trace
            )
            break
        except Exception:
            if attempt == 2:
                raise
            import time as _time

            _time.sleep(10.0)
    global last_results
    last_results = res

    h2 = np.concatenate([res.results[cr]["out"] for cr in range(NC)], axis=0)[:N]
    out = np.zeros((pad_n_i, D), np.float32)
    out[pos_idx] = h2
    return out


# revision 17
# speedup vs baseline: 1.2643x; 1.2477x over previous
"""GCN encoder (2-layer graph conv) on 8 Trainium2 NeuronCores.

Sharding: nodes (rows of x / output) by contiguous 6272-row blocks across the
8 cores; edges partitioned by destination row; 256x256 weights replicated;
per-layer fp16 AllGather of the dense support into G=3 source-group tables
(31/9/9 dest-tile split keeps table row ids int16 AND lets the last AllGather
piece be small so the layer-2 transition bubble shrinks); per-128-dest-tile
segment-sum as scaled-one-hot selector matmuls over dma_gathered source rows.

v2 changes (driven by the v1 trace: DMA queues ~78% busy were the limiter,
sel loads alone were ~106MB = ~20% of DMA traffic; GpSimd "busy" was mostly
SWDGE FIFO backpressure, not descriptor-gen compute):
  - selector chunks are generated ON DEVICE by the (idle) Vector engine:
    one tensor_scalar(is_equal, mult) per chunk over a constant iota row,
    with per-partition (dest_id, val) scalar columns uploaded once per graph
    (2 x [128, TOTC] fp16 ~ 0.9MB instead of 106MB of dense selectors).
    This requires one edge per selector lane, so host-side source dedup is
    dropped (+~4% gather indices).
  - G=3 source groups: AllGather pieces fire as their dense/agg tiles
    complete; the last piece is 9 tiles (4.7MB) instead of 18 (9.4MB).
  - initial ring prefetch right after each AllGather piece is issued, so
    gathers start the moment their table lands.
"""

import os
import sys

if "/opt/trn_rl_repo" not in sys.path:
    sys.path.insert(0, "/opt/trn_rl_repo")

import numpy as np

import concourse.mybir as mybir
import concourse.tile as tile
from concourse import bacc, bass_utils
from concourse.bass import ts
from concourse.library_config import mlp

N = 50000
D = 256
NC = 8
P = 128
T = 49
SHARD = T * P  # 6272
NPAD = NC * SHARD  # 50176

G = 3
GTILES = (31, 9, 9)             # dest tiles per source group
GT = (0, 31, 40, 49)            # tile boundaries
GB = (0, 31 * P, 40 * P, 49 * P)  # per-core row boundaries (3968, 5120, 6272)
GR = (31 * P, 9 * P, 9 * P)     # rows/core per group
TROWS = tuple(NC * r for r in GR)  # table rows: 31744, 9216, 9216 (< 32768)

RINGS = (96, 40, 40)  # ring slots (chunks) per source group; must comfortably
                      # exceed the in-flight window ((PREF+1) tiles of chunks +
                      # OPC overshoot) or gathers serialize against PE reads
OPC = 8               # chunks per dma_gather op (1024 idxs)

F16 = mybir.dt.float16
F32 = mybir.dt.float32
I16 = mybir.dt.int16

# gather-table dtype: F16 (safe, rel_max ~3e-4) or float8e4 (halves gather/AG
# bytes; measured rel_max 0.0148 on the fixed inputs, under the 2e-2 gate).
TAB = mybir.dt.float8e4
TAB_BYTES = 2 if TAB == F16 else 1

_cache: dict = {}
last_results = None


def _build(cgt: tuple):
    """cgt: T-tuple of G-tuples of per-(tile, group) chunk counts."""
    key = cgt
    if key in _cache:
        return _cache[key]

    ct = [sum(c) for c in cgt]
    off = np.concatenate(([0], np.cumsum(ct))).astype(int)  # sel chunk offsets
    goff = []  # per-group cumulative chunk stream offsets over tiles
    for g in range(G):
        goff.append(
            np.concatenate(([0], np.cumsum([cgt[t][g] for t in range(T)]))).astype(int)
        )
    TOTC = int(off[-1])
    TOTG = [int(goff[g][-1]) for g in range(G)]
    TOT = sum(TOTG)
    GBASE = [8 * sum(TOTG[:g]) for g in range(G)]  # gidx column bases
    MAXCT = max(ct)

    nc = bacc.Bacc(
        "TRN2",
        target_bir_lowering=False,
        debug=False,
        num_devices=NC,
        num_swdge_queues=4,
    )

    xT_d = nc.dram_tensor("xT", [2, P, SHARD], F16, kind="ExternalInput")
    w1_d = nc.dram_tensor("W1h", [2, P, D], F16, kind="ExternalInput")
    w2_d = nc.dram_tensor("W2h", [2, P, D], F16, kind="ExternalInput")
    b1_d = nc.dram_tensor("b1c", [P, 2], F32, kind="ExternalInput")
    b2_d = nc.dram_tensor("b2b", [P, D], F32, kind="ExternalInput")
    gidx_d = nc.dram_tensor("gidx", [P, TOT * 8], I16, kind="ExternalInput")
    dst_d = nc.dram_tensor("dst", [P, TOTC], F16, kind="ExternalInput")
    val_d = nc.dram_tensor("val", [P, TOTC], F16, kind="ExternalInput")
    out_d = nc.dram_tensor("out", [SHARD, D], F32, kind="ExternalOutput")

    nc.gpsimd.load_library(mlp)

    rg = [list(range(NC))]

    with tile.TileContext(nc) as tc:
        with (
            tc.tile_pool(name="const", bufs=1) as const,
            tc.tile_pool(name="ring", bufs=1) as ring,
            tc.tile_pool(name="spool", bufs=3) as spool,
            tc.tile_pool(name="dense", bufs=3) as dense,
            tc.tile_pool(name="psD", bufs=3, space="PSUM") as psD,
            tc.tile_pool(name="psA", bufs=2, space="PSUM") as psA,
            tc.tile_pool(name="dram", bufs=1, space="DRAM") as dram,
        ):
            cc1 = [dram.tile([GR[g], D], TAB, name=f"cc1_{g}") for g in range(G)]
            t1 = [
                dram.tile([TROWS[g], D], TAB, addr_space="Shared", name=f"t1_{g}")
                for g in range(G)
            ]
            cc2 = [dram.tile([GR[g], D], TAB, name=f"cc2_{g}") for g in range(G)]
            t2 = [
                dram.tile([TROWS[g], D], TAB, addr_space="Shared", name=f"t2_{g}")
                for g in range(G)
            ]

            gidx = const.tile([P, TOT * 8], I16)
            nc.sync.dma_start(gidx[:], gidx_d[:])
            dst = const.tile([P, TOTC], F16)
            nc.sync.dma_start(dst[:], dst_d[:])
            val = const.tile([P, TOTC], F16)
            nc.sync.dma_start(val[:], val_d[:])
            b1 = const.tile([P, 2], F32)
            nc.sync.dma_start(b1[:], b1_d[:])
            b2 = const.tile([P, D], F32)
            nc.sync.dma_start(b2[:], b2_d[:])
            w1 = const.tile([P, 2 * D], F16)
            w2 = const.tile([P, 2 * D], F16)
            xsb = const.tile([P, 2 * SHARD], F16, tag="xht", name="xsb")
            hT = const.tile([P, 2 * SHARD], F16, tag="xht", name="hT")
            for h in range(2):
                nc.sync.dma_start(w1[:, h * D : (h + 1) * D], w1_d[h])
                nc.sync.dma_start(w2[:, h * D : (h + 1) * D], w2_d[h])
                nc.sync.dma_start(xsb[:, h * SHARD : (h + 1) * SHARD], xT_d[h])

            iot = const.tile([P, P], F16, name="iota")
            nc.gpsimd.iota(
                iot[:],
                [[1, P]],
                channel_multiplier=0,
                allow_small_or_imprecise_dtypes=True,
            )

            rings = [
                ring.tile([P, RINGS[g], D], TAB, name=f"ring{g}") for g in range(G)
            ]

            # per-tile matmul consumption map: k -> (group, chunk-in-stream)
            kmap = []
            for t in range(T):
                m = []
                for g in range(G):
                    for j in range(cgt[t][g]):
                        m.append((g, int(goff[g][t]) + j))
                kmap.append(m)

            def dense_tile(src_sb, w_sb, cc, t):
                ps = psD.tile([P, D], F32, tag="psD", name="ps_dense")
                for h in range(2):
                    nc.tensor.matmul(
                        ps,
                        lhsT=src_sb[:, h * SHARD + t * P : h * SHARD + (t + 1) * P],
                        rhs=w_sb[:, h * D : (h + 1) * D],
                        start=(h == 0),
                        stop=(h == 1),
                    )
                st = dense.tile([P, D], TAB, tag="stage", name="stage")
                nc.scalar.copy(st[:], ps[:])
                g = next(gg for gg in range(G) if GT[gg] <= t < GT[gg + 1])
                nc.sync.dma_start(cc[g][ts(t - GT[g], P), :], st[:])

            def ag(cc_g, t_out):
                nc.gpsimd.collective_compute(
                    "AllGather",
                    mybir.AluOpType.bypass,
                    replica_groups=rg,
                    ins=[cc_g.opt()],
                    outs=[t_out.opt()],
                )

            qctr = [0]

            def make_streams(tabs):
                state = [
                    {"next": 0, "tot": TOTG[g], "table": tabs[g], "ring": rings[g],
                     "base": GBASE[g], "rsz": RINGS[g]}
                    for g in range(G)
                ]

                def ensure(g, upto):
                    s = state[g]
                    while s["next"] < min(upto, s["tot"]):
                        a = s["next"]
                        n = min(OPC, s["tot"] - a)
                        nc.gpsimd.dma_gather(
                            s["ring"][:, a % s["rsz"] : a % s["rsz"] + n, :],
                            s["table"][:],
                            gidx[:, s["base"] + a * 8 : s["base"] + (a + n) * 8],
                            num_idxs=n * P,
                            num_idxs_reg=n * P,
                            elem_size=D,
                            queue_num=qctr[0] % 4,
                        )
                        qctr[0] += 1
                        s["next"] = a + n
                return ensure

            PREF = 2  # prefetch horizon in tiles

            def sel_tile(t):
                c = ct[t]
                o = int(off[t])
                s = spool.tile([P, MAXCT, P], F16, tag="sel", name="sel")
                nc.vector.tensor_tensor(
                    s[:, :c, :],
                    iot[:, None, :].broadcast_to([P, c, P]),
                    dst[:, o : o + c, None].broadcast_to([P, c, P]),
                    mybir.AluOpType.is_equal,
                )
                nc.vector.tensor_tensor(
                    s[:, :c, :],
                    s[:, :c, :],
                    val[:, o : o + c, None].broadcast_to([P, c, P]),
                    mybir.AluOpType.mult,
                )
                return s

            # ---------- layer 1 dense (+ pipelined AllGather pieces) -------
            ensure1 = make_streams(t1)
            for t in range(T):
                dense_tile(xsb, w1, cc1, t)
                for g in range(G):
                    if t == GT[g + 1] - 1:
                        ag(cc1[g], t1[g])
                        ensure1(g, RINGS[g])  # prefetch as soon as table lands

            # ---------- layer 1 agg (+ interleaved layer-2 dense) ----------
            for t in range(T):
                tp = min(t + PREF, T - 1)
                for g in range(G):
                    ensure1(g, int(goff[g][tp + 1]))
                s = sel_tile(t)
                pss = [
                    psA.tile([P, P], F32, tag=f"agg{h}", name=f"agg{h}")
                    for h in range(2)
                ]
                c = ct[t]
                for k, (g, j) in enumerate(kmap[t]):
                    sl = j % RINGS[g]
                    for h in range(2):
                        nc.tensor.matmul(
                            pss[h],
                            lhsT=rings[g][:, sl, h * P : (h + 1) * P],
                            rhs=s[:, k, :],
                            start=(k == 0),
                            stop=(k == c - 1),
                        )
                for h in range(2):
                    nc.scalar.activation(
                        hT[:, h * SHARD + t * P : h * SHARD + (t + 1) * P],
                        pss[h][:],
                        mybir.ActivationFunctionType.Relu,
                        bias=b1[:, h : h + 1],
                    )
                dense_tile(hT, w2, cc2, t)
                for g in range(G):
                    if t == GT[g + 1] - 1:
                        ag(cc2[g], t2[g])

            # ---------- layer 2 ----------
            ensure2 = make_streams(t2)
            for t in range(T):
                tp = min(t + PREF, T - 1)
                for g in range(G):
                    ensure2(g, int(goff[g][tp + 1]))
                s = sel_tile(t)
                ps = psD.tile([P, D], F32, tag="psD", name="agg2")
                c = ct[t]
                for k, (g, j) in enumerate(kmap[t]):
                    sl = j % RINGS[g]
                    nc.tensor.matmul(
                        ps,
                        lhsT=s[:, k, :],
                        rhs=rings[g][:, sl, :],
                        start=(k == 0),
                        stop=(k == c - 1),
                    )
                nc.vector.tensor_tensor(ps[:], ps[:], b2[:], mybir.AluOpType.add)
                ot = dense.tile([P, D], F32, tag="ot", name="ot")
                nc.scalar.activation(ot[:], ps[:], mybir.ActivationFunctionType.Relu)
                nc.sync.dma_start(out_d[ts(t, P), :], ot[:])

    nc.compile()
    _cache[key] = nc
    return nc


def _wrap_idx16(flat: np.ndarray) -> np.ndarray:
    """[L] int -> [128, L/16] int16 SBUF wrap: sb[p, s] = flat[s*16 + p%16]."""
    L = flat.shape[0]
    base = flat.reshape(L // 16, 16).T.astype(np.int16)
    return np.tile(base, (8, 1))


def _preprocess(adj_rows, adj_cols, adj_vals):
    r = np.asarray(adj_rows).astype(np.int64)
    c = np.asarray(adj_cols).astype(np.int64)
    v = np.asarray(adj_vals).astype(np.float32)

    core = r // SHARD
    tile_id = (r % SHARD) // P
    dest_local = r % P
    s_core = c // SHARD
    s_loc = c % SHARD
    grp = (s_loc >= GB[1]).astype(np.int64) + (s_loc >= GB[2]).astype(np.int64)
    idx_local = s_core * np.asarray(GR)[grp] + (s_loc - np.asarray(GB)[grp])

    # slot assignment: rank within each (core, tile, group), edge order
    key = ((core * T + tile_id) * G + grp)
    order = np.argsort(key, kind="stable")
    key_s = key[order]
    first = np.ones(order.shape[0], bool)
    first[1:] = key_s[1:] != key_s[:-1]
    idx = np.arange(order.shape[0])
    start = idx[first]
    gid = np.cumsum(first) - 1
    slot = idx - start[gid]

    cnt = np.zeros(NC * T * G, np.int64)
    np.add.at(cnt, key, 1)
    cnt = cnt.reshape(NC, T, G)
    cgt = tuple(
        tuple(max(1, -(-int(cnt[:, t, g].max()) // P)) for g in range(G))
        for t in range(T)
    )

    ct = [sum(cg) for cg in cgt]
    off = np.concatenate(([0], np.cumsum(ct))).astype(int)
    goff = []
    for g in range(G):
        goff.append(
            np.concatenate(([0], np.cumsum([cgt[t][g] for t in range(T)]))).astype(int)
        )
    TOTC = int(off[-1])
    TOTG = [int(goff[g][-1]) for g in range(G)]
    TOT = sum(TOTG)

    core_s = core[order]
    tile_s = tile_id[order]
    grp_s = grp[order]
    dest_s = dest_local[order]
    v_s = v[order]
    idx_s = idx_local[order]

    chunk_in_tg = slot // P
    lane = slot % P
    # sel column: off[t] + chunks of lower groups + chunk_in_tg
    gprefix = np.zeros((T, G), np.int64)
    for t in range(T):
        acc = 0
        for g in range(G):
            gprefix[t, g] = acc
            acc += cgt[t][g]
    scol = off[tile_s] + gprefix[tile_s, grp_s] + chunk_in_tg

    dst = np.zeros((NC, P, TOTC), np.float16)
    valq = np.zeros((NC, P, TOTC), np.float16)
    dst[core_s, lane, scol] = dest_s.astype(np.float16)
    valq[core_s, lane, scol] = v_s.astype(np.float16)

    # gather index streams, group-concatenated
    stream_base = np.asarray([sum(TOTG[:g]) for g in range(G)])
    stream_chunk = stream_base[grp_s] + goff_lookup(goff, tile_s, grp_s) + chunk_in_tg
    idx_pad = np.zeros((NC, TOT, P), np.int16)
    idx_pad[core_s, stream_chunk, lane] = idx_s.astype(np.int16)
    gidx = np.zeros((NC, P, TOT * 8), np.int16)
    for cr in range(NC):
        gidx[cr] = _wrap_idx16(idx_pad[cr].reshape(-1))

    return cgt, gidx, dst, valq


def goff_lookup(goff, tile_s, grp_s):
    tab = np.stack([goff[g][:T] for g in range(G)], axis=1)  # [T, G]
    return tab[tile_s, grp_s]


def kernel(
    x, adj_rows, adj_cols, adj_vals, pad_n, pos_idx, W1, b1, W2, b2
) -> np.ndarray:
    x = np.asarray(x, np.float32)
    W1 = np.asarray(W1, np.float32)
    b1 = np.asarray(b1, np.float32)
    W2 = np.asarray(W2, np.float32)
    b2 = np.asarray(b2, np.float32)
    pos_idx = np.asarray(pos_idx).astype(np.int64)
    pad_n_i = int(pad_n)
    assert x.shape == (N, D)

    cgt, gidx, dstq, valq = _preprocess(adj_rows, adj_cols, adj_vals)
    nc = _build(cgt)

    xpad = np.zeros((NPAD, D), np.float32)
    xpad[:N] = x
    w1h = W1.astype(np.float16).reshape(2, P, D)
    w2h = W2.astype(np.float16).reshape(2, P, D)
    b1c = np.ascontiguousarray(b1.reshape(2, P).T.astype(np.float32))
    b2b = np.ascontiguousarray(np.broadcast_to(b2, (P, D)).astype(np.float32))

    in_maps = []
    for cr in range(NC):
        xT = np.ascontiguousarray(
            xpad[cr * SHARD : (cr + 1) * SHARD].T.astype(np.float16).reshape(2, P, SHARD)
        )
        in_maps.append(
            {
                "xT": xT,
                "W1h": w1h,
                "W2h": w2h,
                "b1c": b1c,
                "b2b": b2b,
                "gidx": np.ascontiguousarray(gidx[cr]),
                "dst": np.ascontiguousarray(dstq[cr]),
                "val": np.ascontiguousarray(valq[cr]),
            }
        )

    trace = bool(int(os.environ.get("KERNEL_TRACE", "0")))
    res = None
    for attempt in range(3):
        try:
            res = bass_utils.run_bass_kernel_spmd(
                nc, in_maps, core_ids=list(range(NC)), trace=trace
            )
            break
        except Exception:
            if attempt == 2:
                raise
            import time as _time

            _time.sleep(10.0)
    global last_results
    last_results = res

    h2 = np.concatenate([res.results[cr]["out"] for cr in range(NC)], axis=0)[:N]
    out = np.zeros((pad_n_i, D), np.float32)
    out[pos_idx] = h2
    return out
